# revision 20
# baseline (speedup 1.0000x reference)
"""Trainium2 Bass kernel for channel-attention (XCA-style) nn.Module.

Per batch (8 batches -> 8 NeuronCores, pure data parallel):
  qkv = w_qkv @ x            (1x1 conv, 192 -> 576 channels)
  qkv = dwconv3x3(qkv)       (depthwise, per-channel 3x3, zero pad)
  q,k,v = split(qkv); per head (4 heads, 48 ch):
  score = softmax((q/||q||) @ (k/||k||)^T * temp)   contracting hw=16384
  out   = w_proj @ (score @ v)

v2 design notes (per core), measured 670.8us (from 764.7us baseline):
 - q,k qkv channels (384) on PE in fp8e4m3 with DoubleRow perf mode:
   K=192 contracted in ONE matmul; v channels (192) in bf16 (fp8 on
   the v path measured 3.8% rel err - over the 2e-2 gate; q,k fp8
   errors wash out through L2-norm + softmax: 0.26%).
 - qkv psum (f32) evicted by ACT in 1024-px ops into ring A (unified
   [128, 5pb, 18rows, 130] tile; row stride 130 = 128 cols + 2 zero
   gap cols so depthwise taps become shifted APs with correct zero
   padding). The eviction applies a per-channel scale = center dw
   weight, and all tap weights are host-rescaled to w_t/w_center:
   the center-tap product is then a free ring VIEW absorbed into the
   first accumulate add (kills one full DVE mul pass). ring B = ring
   A shifted +1 element (SBUF->SBUF DMA) so dx=+-1 taps stay 4-byte
   aligned for the DVE 4x tensor_scalar mode. Chunk halo rows 0..1
   are DMA-copied from the previous chunk's rows 16..17 instead of
   recomputed (-11% PE qkv / ACT evict / x DMA).
 - depthwise 3x3 split across engines: DVE does 5 tap products
   (tensor_scalar bf16 4x, full-chunk ops) + all 8 accumulate adds
   (tensor_tensor 2x, in-place into ACC, operands merged across the
   5 channel pblocks -> [128, 5, 16, 128] APs); ACT does 3 corner tap
   products (1x, half-chunk granularity to bound SBUF) + norms
   (Square+accum_out) + qt/kt psum evictions. GPSIMD can't help: it
   shares its SBUF port with DVE. Measured dead ends: STT fused MAC
   is 1x (8752ns vs mul 2353 + add 4425 at [128,8192]); tensor_tensor
   has no 4x uop, so the 8 add passes (~350us) are the DVE floor.
 - per-chunk tail work (norms, q/k transposes, qt/kt evictions, score
   matmuls) is software-pipelined one chunk behind the dw so ACT/PE
   never sit between DVE adds and the next chunk's evictions.
 - q,k chunklets transposed on PE (bf16 is_transpose, 4 chunklets per
   psum tile); score^T accumulated over all 128 chunklets in 2
   persistent psum banks.
 - L2 norms + temperature folded into score eviction; full-row softmax
   with -1e30 off-diag mask; AV and proj fused into ONE matmul:
   M^T = P^T @ Wp^T (two tiny [96,192] matmuls on the probs), then
   out = M @ v streamed over hw -> the AV matmuls and the attn
   eviction pass disappear. Output stored bf16, host casts f32.
 - v spilled to DRAM bf16 between stages to fit SBUF.
 - Engine busy at 670us span: ACT 517, DVE 497, PE 291; stage A is
   DVE/ACT-balanced ~100%/85% with a ~95us PE-bound stage-C tail.
"""

import sys

sys.path.insert(0, "/opt/trn_rl_repo")

import numpy as np
import ml_dtypes

import concourse.bass as bass
import concourse.mybir as mybir
import concourse.tile as tile
from concourse import bacc
from concourse.bass import ts, ds
from concourse.bass_utils import run_bass_kernel_spmd
from concourse.masks import make_identity

F32 = mybir.dt.float32
BF16 = mybir.dt.bfloat16
FP8 = mybir.dt.float8e4

DIM = 192
NH = 4
CH = DIM // NH  # 48
C3 = 3 * DIM  # 576
CQK = 384  # q,k output channels (fp8 path)
H = 128
W = 128
HW = H * W
B = 8

NPB = 5  # qkv channel partition blocks: 4x128 + 64
PB_SZ = [128, 128, 128, 128, 64]
CHUNK = 16  # image rows per chunk
NCHUNK = H // CHUNK
RROWS = CHUNK + 2  # ring rows = chunk + halo
RSTR = 130  # ring row stride in elements (128 + 2 zero gap)
NTPC = (W * CHUNK) // 128  # 128-px chunklets per chunk

MUL = mybir.AluOpType.mult
ADD = mybir.AluOpType.add
AF = mybir.ActivationFunctionType
AX = mybir.AxisListType
DR = mybir.MatmulPerfMode.DoubleRow

# tap index i = 3*(dy+1) + (dx+1)
ALL_TAPS = [(dy, dx) for dy in (-1, 0, 1) for dx in (-1, 0, 1)]
ACT_TAPS = [(-1, -1), (-1, 1), (1, 1)]  # corner taps on ACT (1x, any align)
DVE_TAPS = [t for t in ALL_TAPS if t not in ACT_TAPS]  # 6 taps, 4x from rings


def build():
    nc = bacc.Bacc(None, target_bir_lowering=False)

    x8d = nc.dram_tensor("x8", [96, 2, HW], FP8, kind="ExternalInput")
    xbd = nc.dram_tensor("xb", [DIM, HW], BF16, kind="ExternalInput")
    wq8d = nc.dram_tensor("wq8", [96, 2, CQK], FP8, kind="ExternalInput")
    wqbd = nc.dram_tensor("wqb", [128, 2, DIM], BF16, kind="ExternalInput")
    wpd = nc.dram_tensor("wp", [96, 2, DIM], BF16, kind="ExternalInput")
    dwd = nc.dram_tensor("dww", [128, NPB, 9], F32, kind="ExternalInput")
    w4d = nc.dram_tensor("w4v", [128, NPB], F32, kind="ExternalInput")
    tvd = nc.dram_tensor("tmpv", [128, 3], F32, kind="ExternalInput")
    mkd = nc.dram_tensor("mask", [96, 96], F32, kind="ExternalInput")
    isd = nc.dram_tensor("idshift", [128, 64], BF16, kind="ExternalInput")
    outd = nc.dram_tensor("out", [DIM, HW], BF16, kind="ExternalOutput")
    # v spill scratch (bf16), head-pair split: ch 0..95 and ch 96..191
    vda = nc.dram_tensor("vsa", [96, HW], BF16, kind="Internal")
    vdb = nc.dram_tensor("vsb", [96, HW], BF16, kind="Internal")

    with tile.TileContext(nc) as tc:
        _body(nc, tc, x8d, xbd, wq8d, wqbd, wpd, dwd, w4d, tvd, mkd, isd,
              outd, vda, vdb)
    nc.compile()
    return nc


def _body(nc, tc, x8d, xbd, wq8d, wqbd, wpd, dwd, w4d, tvd, mkd, isd, outd,
          vda, vdb):
    import contextlib

    with contextlib.ExitStack() as ctx:
        consts = ctx.enter_context(tc.tile_pool(name="consts", bufs=1))
        smx = ctx.enter_context(tc.tile_pool(name="smx", bufs=1))

        # ---------------- constants ----------------
        wq8 = consts.tile([96, 2, CQK], FP8, tag="wq8")
        nc.sync.dma_start(wq8[:], wq8d[:, :, :])
        wqb = consts.tile([128, 2, DIM], BF16, tag="wqb")
        nc.sync.dma_start(wqb[:], wqbd[:, :, :])
        wp = consts.tile([96, 2, DIM], BF16, tag="wp")
        nc.sync.dma_start(wp[:], wpd[:, :, :])
        dww = consts.tile([128, NPB, 9], F32, tag="dww")
        nc.sync.dma_start(dww[:], dwd[:, :, :])
        w4t = consts.tile([128, NPB], F32, tag="w4t")
        nc.sync.dma_start(w4t[:], w4d[:, :])
        tmpv = consts.tile([128, 3], F32, tag="tmpv")
        nc.sync.dma_start(tmpv[:], tvd[:, :])
        mask = consts.tile([96, 96], F32, tag="mask")
        nc.sync.dma_start(mask[:], mkd[:, :])
        ident = consts.tile([128, 128], F32, tag="ident")
        make_identity(nc, ident[:])
        identb = consts.tile([128, 128], BF16, tag="identb")
        make_identity(nc, identb[:])
        idsh = consts.tile([128, 64], BF16, tag="idsh")
        nc.sync.dma_start(idsh[:], isd[:, :])
        n2 = consts.tile([128, 3], F32, tag="n2")
        nc.vector.memset(n2[:], 0.0)

        # ============ stage A: qkv + dw + norms + score^T ============
        with contextlib.ExitStack() as sa:
            ringp = sa.enter_context(tc.tile_pool(name="ring", bufs=2))
            xp = sa.enter_context(tc.tile_pool(name="xp", bufs=2))
            xp8 = sa.enter_context(tc.tile_pool(name="xp8", bufs=1))
            accp = sa.enter_context(tc.tile_pool(name="accp", bufs=2))
            pap = sa.enter_context(tc.tile_pool(name="pap", bufs=1))
            qp = sa.enter_context(tc.tile_pool(name="qp", bufs=2))
            nrm = sa.enter_context(tc.tile_pool(name="nrm", bufs=2))
            qtp = sa.enter_context(tc.tile_pool(name="qtp", bufs=1))
            pssc = sa.enter_context(
                tc.tile_pool(name="pssc", bufs=1, space=bass.MemorySpace.PSUM)
            )
            sb = contextlib.ExitStack()
            psqkv = sb.enter_context(
                tc.tile_pool(name="psqkv", bufs=2, space=bass.MemorySpace.PSUM)
            )
            tpsp = sb.enter_context(
                tc.tile_pool(name="tps", bufs=1, space=bass.MemorySpace.PSUM)
            )

            scps = [
                pssc.tile([96, 96], F32, tag=f"sc{i}", name=f"scps{i}")
                for i in range(2)
            ]

            sqd = consts.tile([128, 1024], BF16, tag="sqd")

            def tail_block(cp, acc):
                # norms + transposes + qt/kt evict + score mms for chunk cp
                for pb in range(3):
                    for hc in range(2):
                        part = nrm.tile([128, 1], F32, tag="part")
                        nc.scalar.activation(
                            sqd[:],
                            acc[:, pb, ds(hc * 1024, 1024)],
                            AF.Square,
                            accum_out=part[:],
                        )
                        nc.vector.tensor_tensor(
                            n2[:, pb : pb + 1], n2[:, pb : pb + 1], part[:],
                            op=ADD,
                        )
                qt = qtp.tile([128, NTPC, DIM], BF16, tag="qt")
                kt = qtp.tile([128, NTPC, DIM], BF16, tag="kt")
                for ii in range(0, NTPC, 4):
                    tq = tpsp.tile([128, 4, DIM], BF16, tag="tq")
                    tk = tpsp.tile([128, 4, DIM], BF16, tag="tk")
                    for u in range(4):
                        i = ii + u
                        nc.tensor.transpose(
                            tq[:, u, 0:128], acc[:, 0, ts(i, 128)], identb[:]
                        )
                        nc.tensor.transpose(
                            tq[:, u, 128:192],
                            acc[0:64, 1, ts(i, 128)],
                            identb[0:64, 0:64],
                        )
                        nc.tensor.transpose(
                            tk[:, u, 0:64],
                            acc[64:128, 1, ts(i, 128)],
                            idsh[64:128, :],
                        )
                        nc.tensor.transpose(
                            tk[:, u, 64:192], acc[:, 2, ts(i, 128)], identb[:]
                        )
                    nc.scalar.copy(qt[:, ii : ii + 4, :], tq[:])
                    nc.scalar.copy(kt[:, ii : ii + 4, :], tk[:])
                for i in range(NTPC):
                    first = cp == 0 and i == 0
                    last = cp == NCHUNK - 1 and i == NTPC - 1
                    nc.tensor.matmul(
                        scps[0][:],
                        kt[:, i, 0:96],
                        qt[:, i, 0:96],
                        start=first,
                        stop=last,
                    )
                    nc.tensor.matmul(
                        scps[1][:],
                        kt[:, i, 96:192],
                        qt[:, i, 96:192],
                        start=first,
                        stop=last,
                    )

            acc_hist = {}
            def build_rings(c, prevA):
                r0 = c * CHUNK - 1  # raw image row held by ring row 0
                ev_lo = 1 if c == 0 else 2  # rows 0..1 come from prev chunk
                ev_hi = RROWS - 1 if c == NCHUNK - 1 else RROWS
                npix = (ev_hi - ev_lo) * W
                base_px = (r0 + ev_lo) * W

                rallA = ringp.tile([128, NPB, RROWS, RSTR], BF16, tag="rallA",
                                   name=f"rallA_{c}")
                rallB = ringp.tile([128, NPB, RROWS, RSTR], BF16, tag="rallB",
                                   name=f"rallB_{c}")
                # zero the gap columns (stale from slot reuse)
                nc.vector.memset(rallA[:, :, :, 128:RSTR], 0.0)
                if c == 0:
                    nc.vector.memset(rallA[:, :, 0, :], 0.0)
                else:
                    # halo rows 0..1 = previous chunk's rows 16..17
                    nc.sync.dma_start(
                        rallA[:, :, 0:2, :], prevA[:, :, CHUNK : CHUNK + 2, :]
                    )
                if c == NCHUNK - 1:
                    nc.vector.memset(rallA[:, :, RROWS - 1, :], 0.0)

                # --- x in (full chunk) + qkv matmul; 1024-px psum tiles ---
                xt8 = xp8.tile([96, 2, RROWS * W], FP8, tag="xt8")
                nc.sync.dma_start(
                    xt8[:, :, :npix], x8d[:, :, ds(base_px, npix)]
                )
                # q,k pblocks: fp8 DoubleRow, one matmul per 512-px group
                for mb in range(3):
                    for w0 in range(0, npix, 1024):
                        wn = min(1024, npix - w0)
                        ps = psqkv.tile([128, 1024], F32, tag="qkvps")
                        for s0 in range(0, wn, 512):
                            sn = min(512, wn - s0)
                            nc.tensor.matmul(
                                ps[:, ds(s0, sn)],
                                wq8[:, :, ts(mb, 128)],
                                xt8[:, :, ds(w0 + s0, sn)],
                                start=True,
                                stop=True,
                                perf_mode=DR,
                            )
                        rr = ev_lo + (w0 // 128)
                        nc.scalar.activation(
                            rallA[:, mb, rr : rr + wn // 128, 0:128],
                            ps[:, :wn].rearrange("p (r w) -> p r w", w=128),
                            AF.Copy,
                            scale=w4t[:, mb : mb + 1],
                        )
                # v pblocks: bf16, stream 1024-px x tiles
                for w0 in range(0, npix, 1024):
                    wn = min(1024, npix - w0)
                    xtb = xp.tile([128, 2, 1024], BF16, tag="xtb")
                    nc.sync.dma_start(
                        xtb[:, 0, :wn], xbd[0:128, ds(base_px + w0, wn)]
                    )
                    nc.sync.dma_start(
                        xtb[0:64, 1, :wn], xbd[128:192, ds(base_px + w0, wn)]
                    )
                    for mb in (3, 4):
                        msz = PB_SZ[mb]
                        vcol = (mb - 3) * 128
                        ps = psqkv.tile([128, 1024], F32, tag="qkvps")
                        for s0 in range(0, wn, 512):
                            sn = min(512, wn - s0)
                            nc.tensor.matmul(
                                ps[:msz, ds(s0, sn)],
                                wqb[:, 0, ds(vcol, msz)],
                                xtb[:, 0, ds(s0, sn)],
                                start=True,
                                stop=False,
                            )
                            nc.tensor.matmul(
                                ps[:msz, ds(s0, sn)],
                                wqb[0:64, 1, ds(vcol, msz)],
                                xtb[0:64, 1, ds(s0, sn)],
                                start=False,
                                stop=True,
                            )
                        rr = ev_lo + (w0 // 128)
                        nc.scalar.activation(
                            rallA[:msz, mb, rr : rr + wn // 128, 0:128],
                            ps[:msz, :wn].rearrange("p (r w) -> p r w", w=128),
                            AF.Copy,
                            scale=w4t[:msz, mb : mb + 1],
                        )

                # --- ring B = ring A shifted one element; fix first elem ---
                nel = NPB * RROWS * RSTR
                av = rallA[:].rearrange("p b r s -> p (b r s)")
                bv = rallB[:].rearrange("p b r s -> p (b r s)")
                nc.sync.dma_start(bv[:, 1:nel], av[:, 0 : nel - 1])
                # on GPSIMD so it can't block the DVE queue behind the DMA
                nc.gpsimd.memset(rallB[:, 0, 0, 0:1], 0.0)
                return rallA, rallB

            ring_hist = {0: build_rings(0, None)}
            for c in range(NCHUNK):
                rallA, rallB = ring_hist.pop(c)

                def tap(pb, dy, dx, hc):
                    psz = PB_SZ[pb]
                    rlo = 1 + dy + 8 * hc
                    if dx == 0:
                        return rallA[:psz, pb, rlo : rlo + 8, 0:128]
                    return rallB[:psz, pb, rlo : rlo + 8, 1 + dx : 129 + dx]

                def ftap(pb, dy, dx):
                    psz = PB_SZ[pb]
                    rlo = 1 + dy
                    if dx == 0:
                        return rallA[:psz, pb, rlo : rlo + CHUNK, 0:128]
                    return rallB[:psz, pb, rlo : rlo + CHUNK, 1 + dx : 129 + dx]

                # ACT corner products first in ACT's queue for this body
                qtiles = {}
                for hc in range(2):
                    for dy, dx in ACT_TAPS:
                        i = 3 * (dy + 1) + (dx + 1)
                        q = qp.tile([128, NPB, 1024], BF16, tag="qprod")
                        for pb in range(NPB):
                            psz = PB_SZ[pb]
                            nc.scalar.activation(
                                q[:psz, pb, :].rearrange(
                                    "p (r w) -> p r w", w=128
                                ),
                                tap(pb, dy, dx, hc),
                                AF.Copy,
                                scale=dww[:psz, pb, i : i + 1],
                            )
                        qtiles[(hc, (dy, dx))] = q

                # producer pipeline: build next chunk's rings while the DVE
                # chews this chunk (evicts land behind the products in ACT's
                # queue; ring B is ready before the next body starts)
                if c + 1 < NCHUNK:
                    ring_hist[c + 1] = build_rings(c + 1, rallA)

                # consumer pipeline: previous chunk's tail work
                if c > 0:
                    tail_block(c - 1, acc_hist.pop(c - 1))

                # --- depthwise 3x3 ---
                acc = accp.tile([128, NPB, CHUNK * W], BF16, tag="acc")
                acc_hist[c] = acc
                # DVE products (full-chunk ops); center tap comes free as a
                # ring view (ring pre-scaled by the center weight)
                acc4 = acc[:].rearrange("p b (r w) -> p b r w", w=128)
                center = rallA[:, :, 1 : 1 + CHUNK, 0:128]
                mul_taps = [t for t in DVE_TAPS if t != (0, 0)]
                first = True
                for dy, dx in mul_taps:
                    i = 3 * (dy + 1) + (dx + 1)
                    pa = pap.tile([128, NPB, CHUNK * W], BF16, tag="pa")
                    for pb in range(NPB):
                        psz = PB_SZ[pb]
                        nc.vector.tensor_scalar(
                            pa[:psz, pb, :].rearrange(
                                "p (r w) -> p r w", w=128
                            ),
                            ftap(pb, dy, dx),
                            dww[:psz, pb, i : i + 1],
                            None,
                            op0=MUL,
                        )
                    if first:
                        nc.vector.tensor_tensor(
                            acc4,
                            center,
                            pa[:].rearrange("p b (r w) -> p b r w", w=128),
                            op=ADD,
                        )
                        first = False
                    else:
                        nc.vector.tensor_tensor(acc[:], acc[:], pa[:], op=ADD)
                for hc in range(2):
                    accv = acc[:, :, ds(hc * 1024, 1024)]
                    for key in ACT_TAPS:
                        q = qtiles[(hc, key)]
                        nc.vector.tensor_tensor(accv, accv, q[:], op=ADD)

                # --- spill v chunk to DRAM (pair-split) ---
                csl = ds(c * CHUNK * W, CHUNK * W)
                nc.sync.dma_start(vda[:, csl], acc[0:96, 3, :])
                nc.sync.dma_start(vdb[0:32, csl], acc[96:128, 3, :])
                nc.sync.dma_start(vdb[32:96, csl], acc[0:64, 4, :])

            tail_block(NCHUNK - 1, acc_hist.pop(NCHUNK - 1))

            sb.close()
            psB = sa.enter_context(
                tc.tile_pool(name="psB", bufs=1, space=bass.MemorySpace.PSUM)
            )
            # ---------- score finalize + softmax ----------
            rs = smx.tile([128, 3], F32, tag="rs")
            nc.scalar.activation(rs[:], n2[:], AF.Sqrt)
            nc.vector.tensor_scalar(
                rs[:], rs[:], 1e-12, None, op0=mybir.AluOpType.max
            )
            nc.vector.reciprocal(rs[:], rs[:])
            nc.vector.tensor_tensor(rs[:], rs[:], tmpv[:], op=MUL)

            # partition-aligned scale vectors for score rows
            rsq_b = smx.tile([96, 1], F32, tag="rsqb")
            rsk_a = smx.tile([96, 1], F32, tag="rska")
            rsk_b = smx.tile([96, 1], F32, tag="rskb")
            nc.sync.dma_start(rsq_b[0:32, :], rs[96:128, 0:1])
            nc.sync.dma_start(rsq_b[32:96, :], rs[0:64, 1:2])
            nc.sync.dma_start(rsk_a[0:64, :], rs[64:128, 1:2])
            nc.sync.dma_start(rsk_a[64:96, :], rs[0:32, 2:3])
            nc.sync.dma_start(rsk_b[:, :], rs[32:128, 2:3])
            rsq_a = rs[:, 0:1]

            sc_t = smx.tile([96, 2, 96], F32, tag="sct")
            nc.scalar.activation(
                sc_t[:, 0, :], scps[0][:], AF.Copy, scale=rsk_a[:]
            )
            nc.scalar.activation(
                sc_t[:, 1, :], scps[1][:], AF.Copy, scale=rsk_b[:]
            )
            scp2 = [
                psB.tile([96, 96], F32, tag=f"sc2_{i}", name=f"scp2_{i}")
                for i in range(2)
            ]
            nc.tensor.transpose(scp2[0][:], sc_t[:, 0, :], ident[0:96, 0:96])
            nc.tensor.transpose(scp2[1][:], sc_t[:, 1, :], ident[0:96, 0:96])

            sc = smx.tile([96, 2, 96], F32, tag="sc")
            for g in range(2):
                qsc = rsq_a[0:96] if g == 0 else rsq_b[0:96]
                nc.scalar.activation(
                    sc[:, g, :], scp2[g][:], AF.Copy, scale=qsc
                )
                nc.vector.tensor_tensor(
                    sc[:, g, :], sc[:, g, :], mask[:], op=ADD
                )

            probs = smx.tile([96, 2, 96], F32, tag="probs")
            for g in range(2):
                mx = smx.tile([96, 1], F32, tag=f"mx{g}", name=f"mx{g}")
                nc.vector.reduce_max(mx[:], sc[:, g, :], axis=AX.X)
                nmx = smx.tile([96, 1], F32, tag=f"nmx{g}", name=f"nmx{g}")
                nc.vector.tensor_scalar(nmx[:], mx[:], -1.0, None, op0=MUL)
                e = smx.tile([96, 96], F32, tag=f"e{g}", name=f"e{g}")
                nc.scalar.activation(e[:], sc[:, g, :], AF.Exp, bias=nmx[:])
                sm = smx.tile([96, 1], F32, tag=f"sm{g}", name=f"sm{g}")
                nc.vector.reduce_sum(sm[:], e[:], axis=AX.X)
                nc.vector.reciprocal(sm[:], sm[:])
                nc.vector.tensor_scalar(
                    probs[:, g, :], e[:], sm[:], None, op0=MUL
                )

            # ---------- fold proj into probs: MT_g = P_g^T @ WpT_g ----------
            pB = smx.tile([96, 2, 96], BF16, tag="pB")
            nc.vector.tensor_copy(pB[:], probs[:])
            psMT = psB.tile([96, 2, DIM], F32, tag="psMT")
            for g in range(2):
                nc.tensor.matmul(
                    psMT[:, g, :], pB[:, g, :], wp[:, g, :],
                    start=True, stop=True,
                )
            MT = smx.tile([96, 2, DIM], BF16, tag="MT")
            nc.scalar.copy(MT[:], psMT[:])

        # ============ stage C: out = M @ v, streaming over hw ============
        with (
            tc.tile_pool(name="psO", bufs=2, space=bass.MemorySpace.PSUM) as psO,
            tc.tile_pool(name="vload", bufs=3) as vload,
            tc.tile_pool(name="outp", bufs=2) as outp,
        ):
            for j in range(HW // 1024):
                sl = ts(j, 1024)
                vla = vload.tile([96, 1024], BF16, tag="vla")
                vlb = vload.tile([96, 1024], BF16, tag="vlb")
                nc.sync.dma_start(vla[:], vda[:, sl])
                nc.sync.dma_start(vlb[:], vdb[:, sl])

                pa = psO.tile([128, 1024], F32, tag="pa")
                pb_ = psO.tile([64, 1024], F32, tag="pb")
                for sub in range(2):
                    ssl = ds(sub * 512, 512)
                    nc.tensor.matmul(
                        pa[:, ssl], MT[:, 0, 0:128], vla[:, ssl],
                        start=True, stop=False,
                    )
                    nc.tensor.matmul(
                        pa[:, ssl], MT[:, 1, 0:128], vlb[:, ssl],
                        start=False, stop=True,
                    )
                    nc.tensor.matmul(
                        pb_[:, ssl], MT[:, 0, 128:192], vla[:, ssl],
                        start=True, stop=False,
                    )
                    nc.tensor.matmul(
                        pb_[:, ssl], MT[:, 1, 128:192], vlb[:, ssl],
                        start=False, stop=True,
                    )
                ot = outp.tile([128, 1024], BF16, tag="ot")
                ot2 = outp.tile([64, 1024], BF16, tag="ot2")
                nc.scalar.copy(ot[:], pa[:])
                nc.scalar.copy(ot2[:], pb_[:])
                nc.sync.dma_start(outd[0:128, sl], ot[:])
                nc.sync.dma_start(outd[128:192, sl], ot2[:])


_NC_CACHE = {}


def _get_nc():
    if "v2" not in _NC_CACHE:
        _NC_CACHE["v2"] = build()
    return _NC_CACHE["v2"]


def prep_inputs(x, w_qkv, w_dw, w_proj, temperature):
    x = np.asarray(x, np.float32)
    w_qkv = np.asarray(w_qkv, np.float32)
    w_dw = np.asarray(w_dw, np.float32).reshape(C3, 9)
    w_proj = np.asarray(w_proj, np.float32)
    temperature = np.asarray(temperature, np.float32).reshape(NH)

    wqT = np.ascontiguousarray(w_qkv.T)  # [192, 576]
    wq8 = np.zeros((96, 2, CQK), np.float32)
    wq8[:, 0, :] = wqT[0:96, 0:CQK]
    wq8[:, 1, :] = wqT[96:192, 0:CQK]
    wq8 = wq8.astype(ml_dtypes.float8_e4m3fn)

    wqb = np.zeros((128, 2, DIM), np.float32)
    wqb[:, 0, :] = wqT[0:128, CQK:C3]
    wqb[0:64, 1, :] = wqT[128:192, CQK:C3]
    wqb = wqb.astype(ml_dtypes.bfloat16)

    wpT = np.ascontiguousarray(w_proj.T)  # [c, o]
    wp = np.zeros((96, 2, DIM), np.float32)
    wp[:, 0, :] = wpT[0:96]
    wp[:, 1, :] = wpT[96:192]
    wp = wp.astype(ml_dtypes.bfloat16)

    # rescale tap weights by the center weight: the ring holds w4*qkv so
    # the center product is a free ring view; other taps use ratios w_t/w4
    w4 = w_dw[:, 4].copy()
    guard = 1e-6 * (np.abs(w_dw).max(axis=1) + 1e-30)
    small = np.abs(w4) < guard
    w4 = np.where(small, np.where(w4 < 0, -guard, guard), w4)
    ratios = w_dw / w4[:, None]
    dww = np.zeros((128, NPB, 9), np.float32)
    w4v = np.zeros((128, NPB), np.float32)
    for pb in range(NPB):
        sz = PB_SZ[pb]
        dww[:sz, pb, :] = ratios[pb * 128 : pb * 128 + sz]
        w4v[:sz, pb] = w4[pb * 128 : pb * 128 + sz]

    idshift = np.zeros((128, 64), np.float32)
    idshift[64:128, :] = np.eye(64)
    idshift = idshift.astype(ml_dtypes.bfloat16)

    mask = np.full((96, 96), -1e30, np.float32)
    mask[0:48, 0:48] = 0.0
    mask[48:96, 48:96] = 0.0

    tmpv = np.ones((128, 3), np.float32)
    tmpv[:, 0] = temperature[np.arange(128) // CH]
    tmpv[0:64, 1] = temperature[(128 + np.arange(64)) // CH]

    maps = []
    for b in range(B):
        xs = np.ascontiguousarray(x[b].reshape(DIM, HW))
        x8 = np.stack([xs[0:96], xs[96:192]], axis=1)  # [96, 2, HW]
        maps.append(
            {
                "x8": np.ascontiguousarray(x8).astype(ml_dtypes.float8_e4m3fn),
                "xb": xs.astype(ml_dtypes.bfloat16),
                "wq8": wq8,
                "wqb": wqb,
                "wp": wp,
                "dww": dww,
                "w4v": w4v,
                "tmpv": tmpv,
                "mask": mask,
                "idshift": idshift,
            }
        )
    return maps


def kernel(x, w_qkv, w_dw, w_proj, temperature, trace=False, tmpdir=None,
           dw_mode=None):
    nc = _get_nc()
    maps = prep_inputs(x, w_qkv, w_dw, w_proj, temperature)
    res = run_bass_kernel_spmd(
        nc, maps, core_ids=list(range(B)), trace=trace, tmpdir=tmpdir
    )
    out = np.stack(
        [
            np.asarray(r["out"]).astype(np.float32).reshape(DIM, H, W)
            for r in res.results
        ]
    )
    kernel.last_exec_time_ns = res.exec_time_ns
    return out


if __name__ == "__main__":
    nc = build()
    print("build ok")
